# revision 17
# baseline (speedup 1.0000x reference)
"""3-layer GCN (nn_GAT_20899310863186) on 8 TRN2 NeuronCores via Bass/Tile.

Strategy (per sharding hint): nodes are row-sharded 6250/core; edges are
partitioned by destination owner and sorted by (src-half, dst-window).
The GCN normalization coef = dis[src]*dis[dst] is factored: AllGather tables
hold dis-prescaled rows (tbl = dis[n]*h[n], bf16), and dis[dst] is applied
once per destination window in the epilogue. Per layer, per core:
  1. dense part on own rows (x @ W1 for L1), scale by dis via the Scalar
     engine's activation(scale=...) during the f32->bf16 copy, AllGather the
     [50000, F] table so every core holds the full gather source in DRAM,
  2. segment-sum over local edges = dma_gather of 128-edge blocks (bf16,
     256B rows) + a pure 0/1 one-hot built on-chip (one DVE tensor_tensor
     is_equal against a broadcast dstw operand; padded slots use dstw=255 so
     their row is all-zero), matmul-accumulated into a PSUM window of 128
     destination nodes,
  3. epilogue per window: (psum_hi + acc_lo + tbl_own) * dis[dst] (+ bias,
     relu); W2/W3 apply post-aggregation via PE-transpose on own rows.
Layer algebra: L1 aggregates x@W1; L2/L3 aggregate h then apply W after.

int16 gather indices only reach 32767, so the node table is split in two
25000-row halves; each (window, half) edge group is padded to 128-edge blocks.
Block counts are maxed across cores so all 8 cores run one SPMD program.
"""

import sys

sys.path.insert(0, "/opt/trn_rl_repo")

import numpy as np
import ml_dtypes

import concourse.bacc as bacc
import concourse.mybir as mybir
import concourse.tile as tile
from concourse import library_config
from concourse.bass_utils import run_bass_kernel_spmd

BF16 = ml_dtypes.bfloat16

N, P = 50000, 8
NSH = N // P                 # 6250 nodes per core
F_IN, H1, H2, C = 256, 128, 64, 16
WIN = (NSH + 127) // 128     # 49 destination windows per core
NPAD = WIN * 128             # 6272
BSPLIT = NSH // 2            # 3125: AG chunk split within each shard
HALF = P * BSPLIT            # 25000 rows per gathered table chunk
CHUNK = 8                    # gather blocks per dma_gather call (ucode ring caps at ~1024 idxs)


def _preprocess(edge_index):
    src = np.asarray(edge_index[0]).astype(np.int64)
    dst = np.asarray(edge_index[1]).astype(np.int64)
    E = src.shape[0]

    deg = (1.0 + np.bincount(dst, minlength=N)).astype(np.float32)
    dis = (1.0 / np.sqrt(deg)).astype(np.float32)

    core = dst // NSH
    dstloc = dst - core * NSH
    win = dstloc >> 7
    dstw = (dstloc & 127).astype(np.float32)
    src_core = src // NSH
    src_off = src - src_core * NSH
    half = (src_off >= BSPLIT).astype(np.int64)
    loc_src = (src_core * BSPLIT + src_off - half * BSPLIT).astype(np.int16)

    cnt = np.zeros((P, 2, WIN), np.int64)
    np.add.at(cnt, (core, half, win), 1)
    B = np.maximum(1, -(-cnt.max(axis=0) // 128))       # [2, WIN] blocks
    Blo, Bhi = int(B[0].sum()), int(B[1].sum())
    BTOT = Blo + Bhi

    blk_base = np.zeros((2, WIN), np.int64)
    blk_base[0] = np.cumsum(B[0]) - B[0]
    blk_base[1] = np.cumsum(B[1]) - B[1]

    order = np.lexsort((loc_src, win, half, core))
    key = (core * 2 + half) * WIN + win
    ks = key[order]
    starts = np.r_[0, np.flatnonzero(np.diff(ks)) + 1]
    gmark = np.zeros(E, np.int64)
    gmark[starts] = 1
    grp = np.cumsum(gmark) - 1
    rank = np.arange(E) - starts[grp]

    c_s, h_s, w_s = core[order], half[order], win[order]
    slot = blk_base[h_s, w_s] * 128 + rank               # within half-stream

    idx_lo = np.zeros((P, Blo * 128), np.int16)
    idx_hi = np.zeros((P, Bhi * 128), np.int16)
    # pad sentinel 255: is_equal against iota 0..127 never fires -> zero row
    dstw_s = np.full((P, BTOT * 128), 255.0, np.float32)

    lo = h_s == 0
    idx_lo[c_s[lo], slot[lo]] = loc_src[order][lo]
    idx_hi[c_s[~lo], slot[~lo]] = loc_src[order][~lo]
    gslot = np.where(lo, slot, Blo * 128 + slot)
    dstw_s[c_s, gslot] = dstw[order]

    def wrap_idx(a):
        n = a.shape[1]
        w = a.reshape(P, n // 16, 16).transpose(0, 2, 1)
        return np.ascontiguousarray(np.tile(w, (1, 8, 1)))

    idx_lo_w = wrap_idx(idx_lo)
    idx_hi_w = wrap_idx(idx_hi)
    # per-block dstw, partition = edge-in-block: [P, 128, BTOT] bf16
    dstw_w = np.ascontiguousarray(
        dstw_s.reshape(P, BTOT, 128).transpose(0, 2, 1)).astype(BF16)

    dis_pad = np.zeros((P, NPAD), np.float32)
    dis_pad[:, :NSH] = dis.reshape(P, NSH)
    dis_w = np.ascontiguousarray(dis_pad.reshape(P, WIN, 128).transpose(0, 2, 1))

    return {
        "B": B, "idx_lo": idx_lo_w, "idx_hi": idx_hi_w,
        "dstw": dstw_w, "dis": dis_w,
    }


def _build(B):
    f32, bf16, i16 = mybir.dt.float32, mybir.dt.bfloat16, mybir.dt.int16
    AO = mybir.AluOpType
    AF = mybir.ActivationFunctionType
    Blo, Bhi = int(B[0].sum()), int(B[1].sum())
    BTOT = Blo + Bhi

    nc = bacc.Bacc("TRN2", num_devices=P, num_swdge_queues=4, dynamic_dma_scratch_size=32768)

    xT_d = nc.dram_tensor("xT", [F_IN, NPAD], bf16, kind="ExternalInput")
    w1_d = nc.dram_tensor("W1", [F_IN, H1], bf16, kind="ExternalInput")
    w2_d = nc.dram_tensor("W2", [H1, H2], f32, kind="ExternalInput")
    w3_d = nc.dram_tensor("W3", [H2, C], f32, kind="ExternalInput")
    b1_d = nc.dram_tensor("b1r", [128, H1], f32, kind="ExternalInput")
    b2_d = nc.dram_tensor("b2r", [128, H2], f32, kind="ExternalInput")
    b3_d = nc.dram_tensor("b3r", [128, C], f32, kind="ExternalInput")
    ds_d = nc.dram_tensor("disw", [128, WIN], f32, kind="ExternalInput")
    id_d = nc.dram_tensor("ident", [128, 128], f32, kind="ExternalInput")
    io_d = nc.dram_tensor("iota", [128, 128], bf16, kind="ExternalInput")
    il_d = nc.dram_tensor("idxlo", [128, Blo * 8], i16, kind="ExternalInput")
    ih_d = nc.dram_tensor("idxhi", [128, Bhi * 8], i16, kind="ExternalInput")
    dw_d = nc.dram_tensor("dstw", [128, BTOT], bf16, kind="ExternalInput")
    out_d = nc.dram_tensor("out", [NSH, C], f32, kind="ExternalOutput")

    from contextlib import ExitStack
    with tile.TileContext(nc) as tc, ExitStack() as est:
        nc.gpsimd.load_library(library_config.mlp)
        gsems = [est.enter_context(nc.semaphore(f"gat{q}")) for q in range(4)]
        with (
            tc.tile_pool(name="const", bufs=1) as const,
            tc.tile_pool(name="dram", bufs=1, space="DRAM") as dram,
            tc.tile_pool(name="xp", bufs=4) as xp,
            tc.tile_pool(name="gat", bufs=16) as gat,
            tc.tile_pool(name="pmp", bufs=12) as pmp,
            tc.tile_pool(name="tmp", bufs=6) as tmp,
            tc.tile_pool(name="pagg", bufs=4, space="PSUM") as pagg,
            tc.tile_pool(name="ptr", bufs=2, space="PSUM") as ptr,
            tc.tile_pool(name="pww", bufs=2, space="PSUM") as pww,
        ):
            ag_in = [
                [dram.tile([BSPLIT, 128], bf16, name=f"agin{l}_{h}")
                 for h in range(2)] for l in range(3)
            ]
            ag_out = [
                [dram.tile([HALF, 128], bf16, addr_space="Shared",
                           name=f"agout{l}_{h}") for h in range(2)]
                for l in range(3)
            ]

            # constants / schedule
            w1_t = const.tile([128, 2, H1], bf16)
            nc.sync.dma_start(w1_t[:], w1_d[:].rearrange("(k p) h -> p k h", p=128))
            w2_t = const.tile([128, H2], f32)
            nc.sync.dma_start(w2_t[:], w2_d[:])
            w3_t = const.tile([H2, C], f32)
            nc.sync.dma_start(w3_t[:], w3_d[:])
            b1_t = const.tile([128, H1], f32)
            nc.sync.dma_start(b1_t[:], b1_d[:])
            b2_t = const.tile([128, H2], f32)
            nc.sync.dma_start(b2_t[:], b2_d[:])
            b3_t = const.tile([128, C], f32)
            nc.sync.dma_start(b3_t[:], b3_d[:])
            ds_t = const.tile([128, WIN], f32)
            nc.sync.dma_start(ds_t[:], ds_d[:])
            idn_t = const.tile([128, 128], f32)
            nc.sync.dma_start(idn_t[:], id_d[:])
            iota_t = const.tile([128, 128], bf16)
            nc.sync.dma_start(iota_t[:], io_d[:])
            il_t = const.tile([128, Blo * 8], i16)
            nc.sync.dma_start(il_t[:], il_d[:])
            ih_t = const.tile([128, Bhi * 8], i16)
            nc.sync.dma_start(ih_t[:], ih_d[:])
            dw_t = const.tile([128, BTOT], bf16)
            nc.sync.dma_start(dw_t[:], dw_d[:])

            tblf = [
                const.tile([128, WIN * 128], bf16, name=f"tbl{l}")
                for l in range(3)
            ]
            nc.vector.memset(tblf[2][:], 0.0)   # L3 table: zero hi 64 cols
            acc = const.tile([128, WIN * 128], f32)

            def ag_rows(w):
                return min(128, NSH - w * 128)

            def table_dma(layer, w, tile_ap):
                r = ag_rows(w)
                base = w * 128
                if base + r <= BSPLIT:
                    nc.sync.dma_start(
                        ag_in[layer][0][base:base + r, :], tile_ap[:r, :])
                elif base >= BSPLIT:
                    nc.sync.dma_start(
                        ag_in[layer][1][base - BSPLIT:base - BSPLIT + r, :],
                        tile_ap[:r, :])
                else:
                    k = BSPLIT - base
                    nc.sync.dma_start(
                        ag_in[layer][0][base:BSPLIT, :], tile_ap[:k, :])
                    nc.sync.dma_start(
                        ag_in[layer][1][0:r - k, :], tile_ap[k:r, :])

            # ---- phase A1 = x @ W1 on own rows, table rows scaled by dis ----
            for w in range(WIN):
                ps = pagg.tile([128, H1], f32, tag="pagg", name="psA1")
                for k in range(2):
                    xt = xp.tile([128, 128], bf16, tag="xt", name="xt")
                    nc.sync.dma_start(
                        xt[:], xT_d[k * 128:(k + 1) * 128, w * 128:(w + 1) * 128]
                    )
                    nc.tensor.matmul(
                        ps[:], xt[:], w1_t[:, k, :], start=(k == 0), stop=(k == 1)
                    )
                wsl = slice(w * 128, (w + 1) * 128)
                nc.scalar.activation(
                    tblf[0][:, wsl], ps[:], AF.Copy, scale=ds_t[:, w:w + 1]
                )
                table_dma(0, w, tblf[0][:, wsl])

            def all_gather(layer, h):
                nc.gpsimd.collective_compute(
                    "AllGather",
                    AO.bypass,
                    replica_groups=[list(range(P))],
                    ins=[ag_in[layer][h].opt()],
                    outs=[ag_out[layer][h].opt()],
                )

            def make_stream(layer, half, fw, inject=()):
                src_ap = ag_out[layer][half][:]
                idx_t = il_t if half == 0 else ih_t
                bw = [int(x) for x in B[half]]
                pass_blocks = sum(bw)
                state = {"tile": None, "base": 0, "nb": 0, "q": 0, "ck": 0,
                         "pb": 0}
                blk0 = 0 if half == 0 else Blo
                inj = dict(inject)

                def g_slice(b):
                    if state["tile"] is None or b >= state["base"] + state["nb"]:
                        nb = min(CHUNK, pass_blocks - b)
                        q = state["q"]
                        t = gat.tile([128, CHUNK, 128], bf16, tag="gat", name="gt")
                        nc.gpsimd.dma_gather(
                            t[:, :nb, :], src_ap, idx_t[:, b * 8:(b + nb) * 8],
                            nb * 128, nb * 128, 128, queue_num=q,
                        )
                        state.update(tile=t, base=b, nb=nb, q=(q + 1) % 4)
                        fn = inj.pop(state["ck"], None)
                        if fn is not None:
                            fn()
                        state["ck"] += 1
                    g = state["tile"][:, b - state["base"], :fw]
                    gb = blk0 + b
                    pm = pmp.tile([128, 128], bf16, tag="pm", name="pm")
                    nc.vector.tensor_tensor(
                        pm[:], iota_t[:],
                        dw_t[:, gb:gb + 1].broadcast_to([128, 128]), AO.is_equal,
                    )
                    return g, pm[:]

                def do_window(w, out_cb):
                    pb = state["pb"]
                    ps = pagg.tile([128, fw], f32, tag="pagg", name="psW")
                    for j in range(bw[w]):
                        g, pm = g_slice(pb + j)
                        nc.tensor.matmul(
                            ps[:], pm, g, start=(j == 0), stop=(j == bw[w] - 1)
                        )
                    out_cb(w, ps)
                    state["pb"] = pb + bw[w]

                return do_window

            def run_layer(layer, lo_cb_, hi_cb_, fw=128, inj_lo=(),
                          inj_hi=(), lead=12):
                lo = make_stream(layer, 0, fw, inj_lo)
                hi = make_stream(layer, 1, fw, inj_hi)
                for w in range(WIN + lead):
                    if w < WIN:
                        lo(w, lo_cb_)
                    if w >= lead:
                        hi(w - lead, hi_cb_)

            def lo_cb(w, ps):
                nc.vector.tensor_copy(acc[:, w * 128:(w + 1) * 128], ps[:])

            def lo_cb64(w, ps):
                nc.vector.tensor_copy(acc[:, w * 128:w * 128 + H2], ps[:])

            def l1_hi(w, ps):
                wsl = slice(w * 128, (w + 1) * 128)
                u = tmp.tile([128, H1], f32, tag="tA", name="u1")
                nc.vector.tensor_tensor(u[:], ps[:], acc[:, wsl], AO.add)
                nc.vector.tensor_tensor(u[:], u[:], tblf[0][:, wsl], AO.add)
                nc.vector.scalar_tensor_tensor(
                    u[:], u[:], ds_t[:, w:w + 1], b1_t[:], AO.mult, AO.add
                )
                nc.scalar.activation(
                    tblf[1][:, wsl], u[:], AF.Relu, scale=ds_t[:, w:w + 1]
                )
                table_dma(1, w, tblf[1][:, wsl])

            def l2_hi(w, ps):
                wsl = slice(w * 128, (w + 1) * 128)
                u = tmp.tile([128, H1], f32, tag="tA", name="u2")
                nc.vector.tensor_tensor(u[:], ps[:], acc[:, wsl], AO.add)
                nc.vector.tensor_tensor(u[:], u[:], tblf[1][:, wsl], AO.add)
                pt = ptr.tile([128, 128], f32, tag="ptr", name="pt2")
                nc.tensor.transpose(pt[:], u[:], idn_t[:])
                uT = tmp.tile([128, 128], f32, tag="tB", name="uT2")
                nc.vector.tensor_copy(uT[:], pt[:])
                pw = pww.tile([128, H2], f32, tag="pw", name="pw2")
                nc.tensor.matmul(pw[:], uT[:], w2_t[:])
                v = tmp.tile([128, H2], f32, tag="tC", name="v2")
                nc.vector.scalar_tensor_tensor(
                    v[:], pw[:], ds_t[:, w:w + 1], b2_t[:], AO.mult, AO.add
                )
                nc.scalar.activation(
                    tblf[2][:, w * 128:w * 128 + H2], v[:], AF.Relu,
                    scale=ds_t[:, w:w + 1],
                )
                table_dma(2, w, tblf[2][:, wsl])

            def l3_hi(w, ps):
                w64 = slice(w * 128, w * 128 + H2)
                v = tmp.tile([128, H2], f32, tag="tC", name="v3")
                nc.vector.tensor_tensor(v[:], ps[:], acc[:, w64], AO.add)
                nc.vector.tensor_tensor(v[:], v[:], tblf[2][:, w64], AO.add)
                pt = ptr.tile([128, 128], f32, tag="ptr", name="pt3")
                nc.tensor.transpose(pt[:H2, :], v[:], idn_t[:])
                vT = tmp.tile([128, 128], f32, tag="tB", name="vT3")
                nc.vector.tensor_copy(vT[:H2, :], pt[:H2, :])
                po = pww.tile([128, C], f32, tag="pw", name="po3")
                nc.tensor.matmul(po[:], vT[:H2, :], w3_t[:])
                o = tmp.tile([128, C], f32, tag="tD", name="o3")
                nc.vector.scalar_tensor_tensor(
                    o[:], po[:], ds_t[:, w:w + 1], b3_t[:], AO.mult, AO.add
                )
                r = ag_rows(w)
                nc.sync.dma_start(out_d[w * 128:w * 128 + r, :], o[:r, :])

            # AG triggers are GpSimd-queue instructions: each layer's hi-chunk
            # AG fires 1 chunk into that layer's (interleaved) stream; the
            # next layer's lo-chunk AG fires once this layer's hi epilogues
            # have covered windows 0..24 (they write ag_in rows < 3125).
            ch24 = [
                -(-sum(int(x) for x in B[1][:25]) // CHUNK) + 2
            ]
            all_gather(0, 0)
            all_gather(0, 1)
            run_layer(0, lo_cb, l1_hi, lead=18,
                      inj_hi={ch24[0]: lambda: all_gather(1, 0)})
            run_layer(1, lo_cb, l2_hi,
                      inj_lo={1: lambda: all_gather(1, 1)},
                      inj_hi={ch24[0]: lambda: all_gather(2, 0)})
            run_layer(2, lo_cb64, l3_hi, fw=H2,
                      inj_lo={1: lambda: all_gather(2, 1)})

    nc.compile()
    return nc


_CACHE = {}
_IOTA = np.ascontiguousarray(
    np.broadcast_to(np.arange(128, dtype=np.float32), (128, 128)).astype(BF16)
)
_IDENT = np.eye(128, dtype=np.float32)


def _make_in_maps(inputs, pre):
    x = np.asarray(inputs["x"], dtype=np.float32)
    W1 = np.asarray(inputs["W1"], dtype=np.float32).astype(BF16)
    b1 = np.asarray(inputs["b1"], dtype=np.float32)
    W2 = np.asarray(inputs["W2"], dtype=np.float32)
    b2 = np.asarray(inputs["b2"], dtype=np.float32)
    W3 = np.asarray(inputs["W3"], dtype=np.float32)
    b3 = np.asarray(inputs["b3"], dtype=np.float32)

    in_maps = []
    for c in range(P):
        xT = np.zeros((F_IN, NPAD), BF16)
        xT[:, :NSH] = x[c * NSH:(c + 1) * NSH].T.astype(BF16)
        in_maps.append({
            "xT": xT,
            "W1": W1, "W2": W2, "W3": W3,
            "b1r": np.ascontiguousarray(np.broadcast_to(b1, (128, H1))),
            "b2r": np.ascontiguousarray(np.broadcast_to(b2, (128, H2))),
            "b3r": np.ascontiguousarray(np.broadcast_to(b3, (128, C))),
            "disw": pre["dis"][c],
            "ident": _IDENT,
            "iota": _IOTA,
            "idxlo": pre["idx_lo"][c],
            "idxhi": pre["idx_hi"][c],
            "dstw": pre["dstw"][c],
        })
    return in_maps


def kernel(**inputs):
    ei = np.asarray(inputs["edge_index"])

    key = hash(ei.tobytes())
    if key not in _CACHE:
        pre = _preprocess(ei)
        nc = _build(pre["B"])
        _CACHE[key] = (nc, pre)
    nc, pre = _CACHE[key]

    in_maps = _make_in_maps(inputs, pre)
    res = run_bass_kernel_spmd(nc, in_maps, core_ids=list(range(P)))
    out = np.concatenate([res.results[c]["out"] for c in range(P)], axis=0)
    return np.ascontiguousarray(out, dtype=np.float32)
